# revision 1
# baseline (speedup 1.0000x reference)
"""Graphormer multi-head attention on 8 TRN2 NeuronCores.

Sharding: heads across cores (H=16 -> 2 heads/core), both batch elements on
every core (B*H = 32 (b,h) units -> 4 per core).

 - QKV projections column-parallel: each core computes only its 2 heads'
   slices (128 of 1024 output dims) from the full inputs.
 - Attention is computed in "transposed" layout: scoresT = (K @ Q^T)*scale
   with S on partitions and T on the free axis.  That makes the PV matmul
   (outT = V^T @ P^T) transpose-free: lhsT = V in natural (S, D) layout,
   rhs = expP in (S, T) layout.
 - The softmax denominator is obtained for free by appending a ones column
   to the V stationary operand (row 64 of the PV accumulator = row sums).
 - exp() has no max-subtraction: logits are O(5) here, no overflow in f32.
 - The two (B,H,T,S) bias tensors + attn_mask (+ key padding) are pre-added
   on the host, transposed to (S,T) layout, and fed per-core as one bf16
   tensor: halves the dominant DMA traffic.
 - Out-projection column-parallel (contract over this core's 128 dims);
   the 8 partial (B,T,E) outputs are summed on the host.

All matmuls run in bf16 with fp32 PSUM accumulation.
"""

import os
from contextlib import ExitStack

import ml_dtypes
import numpy as np

import concourse.bass as bass
import concourse.tile as tile
from concourse import bacc
from concourse import mybir
from concourse.bass_utils import run_bass_kernel_spmd
from concourse.masks import make_identity

B, T, S, E, H, D = 2, 2048, 2048, 1024, 16, 64
NCORES = 8
HPC = H // NCORES          # heads per core = 2
PSL = HPC * D              # per-core projection slice = 128
NB = 512                   # fp32 psum bank free size
BF16 = mybir.dt.bfloat16
F32 = mybir.dt.float32
NPBF16 = ml_dtypes.bfloat16

_MODULES = {}
LAST_RUN = None


def _chunks(total, step):
    out = []
    o = 0
    while o < total:
        w = min(step, total - o)
        out.append((o, w))
        o += w
    return out


def build_module(t=T, s=S):
    key = (t, s)
    if key in _MODULES:
        return _MODULES[key]

    e = E
    EC = e // 128              # contraction chunks for projections
    ST = s // 128              # key tiles
    tch = min(1024, t)         # stage-B T block
    NTH = t // tch

    nc = bacc.Bacc("TRN2", target_bir_lowering=False, debug=False)

    qT_d = nc.dram_tensor("qT", [B, e, t], BF16, kind="ExternalInput")
    kT_d = nc.dram_tensor("kT", [B, e, s], BF16, kind="ExternalInput")
    vT_d = nc.dram_tensor("vT", [B, e, s], BF16, kind="ExternalInput")
    bias_d = nc.dram_tensor("biasT", [B * HPC, s, t], BF16, kind="ExternalInput")
    wq_d = nc.dram_tensor("wqT", [e, PSL], BF16, kind="ExternalInput")
    wk_d = nc.dram_tensor("wkT", [e, PSL], BF16, kind="ExternalInput")
    wv_d = nc.dram_tensor("wvT", [e, PSL], BF16, kind="ExternalInput")
    wo_d = nc.dram_tensor("woT", [PSL, e], BF16, kind="ExternalInput")
    bq_d = nc.dram_tensor("bq", [PSL, 1], F32, kind="ExternalInput")
    bv_d = nc.dram_tensor("bv", [PSL, 1], F32, kind="ExternalInput")
    y_d = nc.dram_tensor("ypart", [B, t, e], F32, kind="ExternalOutput")

    with tile.TileContext(nc) as tc, ExitStack() as ctx:
        consts = ctx.enter_context(tc.tile_pool(name="consts", bufs=1))
        persist = ctx.enter_context(tc.tile_pool(name="persist", bufs=1))
        xpool = ctx.enter_context(tc.tile_pool(name="xstage", bufs=2))
        biasp = ctx.enter_context(tc.tile_pool(name="biasp", bufs=6))
        ptpool = ctx.enter_context(tc.tile_pool(name="ptpool", bufs=4))
        normp = ctx.enter_context(tc.tile_pool(name="normp", bufs=3))
        ysbp = ctx.enter_context(tc.tile_pool(name="ysbp", bufs=2))
        # one PSUM pool, two tags, 2 bufs each: 2*(2+2) banks = 8 banks exactly
        psum = ctx.enter_context(tc.tile_pool(name="psum", bufs=2, space="PSUM"))

        ident = consts.tile([128, 128], BF16, tag="ident", name="ident")
        make_identity(nc, ident[:])
        bq_s = consts.tile([PSL, 1], F32, tag="bq", name="bq")
        nc.sync.dma_start(bq_s[:], bq_d[:])
        bv_s = consts.tile([PSL, 1], F32, tag="bv", name="bv")
        nc.sync.dma_start(bv_s[:], bv_d[:])
        w_sb = {}
        for nm, wd in (("q", wq_d), ("k", wk_d), ("v", wv_d)):
            w_s = consts.tile([128, EC * 128], BF16, tag=f"w{nm}", name=f"w{nm}")
            for ec in range(EC):
                nc.sync.dma_start(w_s[:, ec * 128:(ec + 1) * 128],
                                  wd[ec * 128:(ec + 1) * 128, :])
            w_sb[nm] = w_s
        wo_s = consts.tile([PSL, e], BF16, tag="wo", name="wo")
        nc.sync.dma_start(wo_s[:], wo_d[:])

        qTs, kTs, vnat, outn = {}, {}, {}, {}
        for bb in range(B):
            qTs[bb] = persist.tile([PSL, t], BF16, tag=f"qTs{bb}", name=f"qTs{bb}")
            kTs[bb] = persist.tile([PSL, s], BF16, tag=f"kTs{bb}", name=f"kTs{bb}")
            vnat[bb] = persist.tile([128, ST, 130], BF16, tag=f"vnat{bb}", name=f"vnat{bb}")
            outn[bb] = persist.tile([PSL, t], BF16, tag=f"outn{bb}", name=f"outn{bb}")

        # ---------------- stage A: projections ----------------
        for bb in range(B):
            for which, (xd, w_s, L) in enumerate(
                ((qT_d, w_sb["q"], t), (kT_d, w_sb["k"], s), (vT_d, w_sb["v"], s))
            ):
                xt = xpool.tile([128, EC, max(t, s)], BF16, tag="xt", name="xt")
                for ec in range(EC):
                    nc.sync.dma_start(xt[:, ec, 0:L], xd[bb, ec * 128:(ec + 1) * 128, :])
                if which == 2:
                    vt = xpool.tile([PSL, s], BF16, tag="vt", name="vt")
                for ch0, chw in _chunks(L, 1024):
                    pp = psum.tile([128, 1024], F32, tag="sc", name="sc")
                    for n0, nw in _chunks(chw, NB):
                        for ec in range(EC):
                            nc.tensor.matmul(
                                pp[:, n0:n0 + nw],
                                w_s[:, ec * 128:(ec + 1) * 128],
                                xt[:, ec, ch0 + n0:ch0 + n0 + nw],
                                start=(ec == 0), stop=(ec == EC - 1),
                            )
                    if which == 0:
                        nc.vector.tensor_scalar_add(
                            qTs[bb][:, ch0:ch0 + chw], pp[:, 0:chw], bq_s[:])
                    elif which == 1:
                        nc.scalar.copy(kTs[bb][:, ch0:ch0 + chw], pp[:, 0:chw])
                    else:
                        nc.vector.tensor_scalar_add(
                            vt[:, ch0:ch0 + chw], pp[:, 0:chw], bv_s[:])
                if which == 2:
                    nc.vector.memset(vnat[bb][:, :, 64:65], 1.0)
                    nc.vector.memset(vnat[bb][:, :, 129:130], 1.0)
                    for st in range(ST):
                        ptp = psum.tile([128, 128], BF16, tag="sc", name="sc")
                        nc.tensor.transpose(
                            ptp[:], vt[:, st * 128:(st + 1) * 128], ident[:])
                        nc.scalar.copy(vnat[bb][:, st, 0:64], ptp[:, 0:64])
                        nc.scalar.copy(vnat[bb][:, st, 65:129], ptp[:, 64:128])

        # ---------------- stage B: attention + out-projection ----------------
        for bb in range(B):
            for th in range(NTH):
                t0 = th * tch
                for hh in range(HPC):
                    p0 = 64 * hh
                    u = bb * HPC + hh
                    pout = psum.tile([65, tch], F32, tag="acc", name="acc")
                    for st in range(ST):
                        psc = psum.tile([128, tch], F32, tag="sc", name="sc")
                        bt = biasp.tile([128, tch], BF16, tag="bias", name="bias")
                        nc.sync.dma_start(
                            bt[:], bias_d[u, st * 128:(st + 1) * 128, t0:t0 + tch])
                        for n0, nw in _chunks(tch, NB):
                            nc.tensor.matmul(
                                psc[:, n0:n0 + nw],
                                kTs[bb][p0:p0 + 64, st * 128:(st + 1) * 128],
                                qTs[bb][p0:p0 + 64, t0 + n0:t0 + n0 + nw],
                                start=True, stop=True,
                            )
                        pt = ptpool.tile([128, tch], BF16, tag="pt", name="pt")
                        nc.scalar.activation(
                            pt[:], psc[:], mybir.ActivationFunctionType.Exp)
                        # bias folded in multiplicatively: host sends exp(bias)
                        nc.vector.tensor_mul(pt[:], pt[:], bt[:])
                        for n0, nw in _chunks(tch, NB):
                            nc.tensor.matmul(
                                pout[:, n0:n0 + nw],
                                vnat[bb][:, st, 65 * hh:65 * hh + 65],
                                pt[:, n0:n0 + nw],
                                start=(st == 0), stop=(st == ST - 1),
                            )
                    # copy the accumulator out fast to free the PSUM slot;
                    # den row staged to a partition-0 tile (partition_broadcast
                    # broadcasts the physical partition 0 of its source)
                    po_s = normp.tile([64, tch], F32, tag="po", name="po")
                    nc.scalar.copy(po_s[:], pout[0:64, :])
                    den_s = normp.tile([1, tch], F32, tag="den", name="den")
                    nc.scalar.copy(den_s[:], pout[64:65, :])
                    rb = normp.tile([64, tch], F32, tag="rb", name="rb")
                    nc.gpsimd.partition_broadcast(rb[:], den_s[:])
                    nc.vector.reciprocal(rb[:], rb[:])
                    nc.vector.tensor_mul(
                        outn[bb][p0:p0 + 64, t0:t0 + tch], po_s[:], rb[:])
                # out-projection for the rows of this T block
                for tt0, _ttw in _chunks(tch, 128):
                    py = psum.tile([128, e], F32, tag="acc", name="acc")
                    for n0, nw in _chunks(e, NB):
                        nc.tensor.matmul(
                            py[:, n0:n0 + nw],
                            outn[bb][:, t0 + tt0:t0 + tt0 + 128],
                            wo_s[:, n0:n0 + nw],
                            start=True, stop=True,
                        )
                    ys = ysbp.tile([128, e], F32, tag="ys", name="ys")
                    nc.scalar.copy(ys[:], py[:])
                    nc.sync.dma_start(y_d[bb, t0 + tt0:t0 + tt0 + 128, :], ys[:])

    nc.compile()
    _MODULES[key] = nc
    return nc


def make_in_maps(query, key, value, spatial_bias, directional_bias,
                 key_padding_mask, attn_mask, Wq, bq, Wk, bk, Wv, bv, Wo, bo,
                 t=T, s=S):
    scale = D ** -0.5
    qT = np.ascontiguousarray(query.transpose(0, 2, 1), dtype=NPBF16)
    kT = np.ascontiguousarray(key.transpose(0, 2, 1), dtype=NPBF16)
    vT = np.ascontiguousarray(value.transpose(0, 2, 1), dtype=NPBF16)
    pad_any = bool(np.any(key_padding_mask))
    in_maps = []
    for c in range(NCORES):
        h0 = c * HPC
        sl = slice(h0 * D, (h0 + HPC) * D)
        bias = spatial_bias[:, h0:h0 + HPC].astype(np.float32) \
            + directional_bias[:, h0:h0 + HPC]
        bias += attn_mask[None, None]
        if pad_any:
            bias = np.where(key_padding_mask[:, None, None, :], -1e30, bias)
        np.exp(bias, out=bias)  # kernel applies bias multiplicatively
        biasT = np.ascontiguousarray(
            bias.transpose(0, 1, 3, 2), dtype=NPBF16).reshape(B * HPC, s, t)
        in_maps.append({
            "qT": qT, "kT": kT, "vT": vT, "biasT": biasT,
            "wqT": np.ascontiguousarray((Wq[sl, :].T * scale), dtype=NPBF16),
            "wkT": np.ascontiguousarray(Wk[sl, :].T, dtype=NPBF16),
            "wvT": np.ascontiguousarray(Wv[sl, :].T, dtype=NPBF16),
            "woT": np.ascontiguousarray(Wo[:, sl].T, dtype=NPBF16),
            "bq": bq[sl].reshape(PSL, 1).astype(np.float32),
            "bv": bv[sl].reshape(PSL, 1).astype(np.float32),
        })
    return in_maps


def _install_ntff_shim():
    """bass_utils' trace path imports antenv.axon_hooks, which this image
    lacks; synthesize it around trn_boot's ctypes NTFF hook."""
    import sys
    import types
    if "antenv.axon_hooks" in sys.modules:
        return
    try:
        import antenv
        from trn_agent_boot.trn_boot import _ntff_profile_via_ctypes
        hook = _ntff_profile_via_ctypes("/opt/axon/libaxon_pjrt.so")
        mod = types.ModuleType("antenv.axon_hooks")
        mod._hook = hook
        mod.get_axon_ntff_profile_hook = lambda: mod._hook
        mod.set_axon_ntff_profile_hook = lambda h: setattr(mod, "_hook", h)
        sys.modules["antenv.axon_hooks"] = mod
        antenv.axon_hooks = mod
    except Exception as exc:  # pragma: no cover
        print("ntff shim unavailable:", exc)


def kernel(**inputs):
    global LAST_RUN
    if os.environ.get("BASS_TRACE"):
        _install_ntff_shim()
    nc = build_module()
    in_maps = make_in_maps(**inputs)
    res = run_bass_kernel_spmd(
        nc, in_maps, core_ids=list(range(NCORES)),
        trace=bool(os.environ.get("BASS_TRACE")),
    )
    LAST_RUN = res
    y = res.results[0]["ypart"].astype(np.float64)
    for c in range(1, NCORES):
        y += res.results[c]["ypart"]
    bo = inputs["bo"]
    if np.any(bo):
        y += bo
    return y.astype(np.float32)



# revision 6
# speedup vs baseline: 1.2022x; 1.2022x over previous
"""Graphormer multi-head attention on 8 TRN2 NeuronCores.

Sharding (2D, data + head parallel): core c -> batch c//4, head-quad c%4
(4 heads per core as 2 pairs).  Per-core DMA: q/k/v only for its batch
(12.6 MB), bias slice 33.5 MB bf16, bf16 partial output 4.2 MB.

 - QKV projections column-parallel per pair (128 of 1024 output dims each).
 - Attention in transposed layout: scoresT = K@Q^T with S on partitions and
   T free.  The two heads of a pair use PE row-tiling (K=64 stationaries at
   partitions 0-63 / 64-127 -> tile_position (0,0)/(64,0)) so their scores
   matmuls can overlap in the PE array.
 - Softmax denominator from a ones column appended to the PV stationary
   (row 64 of the PV accumulator); 1/den via the custom-DVE
   reciprocal_approx_fast, broadcast across partitions on idle GPSIMD.
 - bias (spatial+directional+attn_mask, exp'd and bf16 on the host) applied
   multiplicatively on DVE at 2x rate over [128, 2048] tiles.
 - Out-projection column-parallel over this core's 256 dims; bf16 partials
   summed on the host (the all-reduce) together with bo.

Emission is software-pipelined: a minimal head (first 1024-column chunk of
the q/k/v projections + first half of the V transposes for pair 0), then
the attention st-loops with the remaining projection/transpose work woven
one-or-two items per st-pair, so the ACT engine (the exp wall, ~16.8M
elements/core ~= 142us) starts early and never starves.
"""

import os
from contextlib import ExitStack

import ml_dtypes
import numpy as np

import concourse.bass as bass
import concourse.tile as tile
from concourse import bacc
from concourse import mybir
from concourse.bass_utils import run_bass_kernel_spmd
from concourse.masks import make_identity

B, T, S, E, H, D = 2, 2048, 2048, 1024, 16, 64
NCORES = 8
HPC = 4                    # heads per core
NPAIR = 2                  # head pairs per core
PSL = HPC * D              # per-core projection slice = 256
EC = E // 128              # contraction chunks = 8
ST = S // 128              # s tiles = 16
TCH = 1024                 # t block
NTH = T // TCH             # 2
NB = 512                   # fp32 psum bank free size
BF16 = mybir.dt.bfloat16
F32 = mybir.dt.float32
NPBF16 = ml_dtypes.bfloat16
AF = mybir.ActivationFunctionType

_MODULES = {}
LAST_RUN = None


def build_module():
    key = "main"
    if key in _MODULES:
        return _MODULES[key]

    nc = bacc.Bacc("TRN2", target_bir_lowering=False, debug=False)

    qT_d = nc.dram_tensor("qT", [E, T], BF16, kind="ExternalInput")
    kT_d = nc.dram_tensor("kT", [E, S], BF16, kind="ExternalInput")
    vT_d = nc.dram_tensor("vT", [E, S], BF16, kind="ExternalInput")
    # host layout: [head, st, 128, th, TCH] (exp'd bias, transposed (s,t))
    bias_d = nc.dram_tensor("biasT", [HPC, ST, 128, NTH, TCH], BF16,
                            kind="ExternalInput")
    wq_d = nc.dram_tensor("wqT", [E, PSL], BF16, kind="ExternalInput")
    wk_d = nc.dram_tensor("wkT", [E, PSL], BF16, kind="ExternalInput")
    wv_d = nc.dram_tensor("wvT", [E, PSL], BF16, kind="ExternalInput")
    wo_d = nc.dram_tensor("woT", [PSL, E], BF16, kind="ExternalInput")
    bq_d = nc.dram_tensor("bq", [128, NPAIR], F32, kind="ExternalInput")
    bk_d = nc.dram_tensor("bk", [128, NPAIR], F32, kind="ExternalInput")
    bv_d = nc.dram_tensor("bv", [128, NPAIR], F32, kind="ExternalInput")
    y_d = nc.dram_tensor("ypart", [T, E], BF16, kind="ExternalOutput")

    with tile.TileContext(nc) as tc, ExitStack() as ctx:
        consts = ctx.enter_context(tc.tile_pool(name="consts", bufs=1))
        xpool = ctx.enter_context(tc.tile_pool(name="xstage", bufs=1))
        persist = ctx.enter_context(tc.tile_pool(name="persist", bufs=1))
        biasp = ctx.enter_context(tc.tile_pool(name="biasp", bufs=3))
        ptpool = ctx.enter_context(tc.tile_pool(name="ptpool", bufs=4))
        normp = ctx.enter_context(tc.tile_pool(name="normp", bufs=1))
        rdenp = ctx.enter_context(tc.tile_pool(name="rdenp", bufs=1))
        ysp = ctx.enter_context(tc.tile_pool(name="ysp", bufs=2))
        # psum: tag sc = [128,1024] f32 (2 banks) x2, tag acc = 2 banks x2
        psum = ctx.enter_context(tc.tile_pool(name="psum", bufs=2, space="PSUM"))

        ident = consts.tile([128, 128], BF16, tag="ident", name="ident")
        make_identity(nc, ident[:])
        w_sb = {}
        for nm, wd in (("q", wq_d), ("k", wk_d), ("v", wv_d)):
            w_s = consts.tile([128, EC, PSL], BF16, tag=f"w{nm}", name=f"w{nm}")
            for ec in range(EC):
                nc.sync.dma_start(w_s[:, ec, :], wd[ec * 128:(ec + 1) * 128, :])
            w_sb[nm] = w_s
        wo_s = consts.tile([128, NPAIR, E], BF16, tag="wo", name="wo")
        for p in range(NPAIR):
            nc.sync.dma_start(wo_s[:, p, :], wo_d[p * 128:(p + 1) * 128, :])
        b_sb = {}
        for nm, bd in (("q", bq_d), ("k", bk_d), ("v", bv_d)):
            b_s = consts.tile([128, NPAIR], F32, tag=f"b{nm}", name=f"b{nm}")
            nc.sync.dma_start(b_s[:], bd[:])
            b_sb[nm] = b_s

        x_sb, x_d = {}, {"q": qT_d, "k": kT_d, "v": vT_d}
        for nm, L in (("q", T), ("k", S), ("v", S)):
            x_sb[nm] = xpool.tile([128, EC, L], BF16, tag=f"x{nm}", name=f"x{nm}")

        qTs = persist.tile([128, NPAIR, T], BF16, tag="qTs", name="qTs")
        kTs = persist.tile([128, NPAIR, S], BF16, tag="kTs", name="kTs")
        # [s-part, pair, st, head-in-pair, d | ones col]
        vnat = persist.tile([128, NPAIR, ST, 2, 65], BF16, tag="vnat", name="vnat")
        outn = persist.tile([128, NPAIR, T], BF16, tag="outn", name="outn")
        vt_sb = persist.tile([128, NPAIR, S], BF16, tag="vt", name="vt")

        def load_x(nm, c0):
            for ec in range(EC):
                nc.sync.dma_start(x_sb[nm][:, ec, c0:c0 + TCH],
                                  x_d[nm][ec * 128:(ec + 1) * 128, c0:c0 + TCH])

        def proj_chunk(nm, p, c0, on_act):
            """project x[nm] cols [c0, c0+1024) for pair p."""
            pp = psum.tile([128, TCH], F32, tag="sc", name="sc")
            for n0 in range(0, TCH, NB):
                for ec in range(EC):
                    nc.tensor.matmul(
                        pp[:, n0:n0 + NB],
                        w_sb[nm][:, ec, p * 128:(p + 1) * 128],
                        x_sb[nm][:, ec, c0 + n0:c0 + n0 + NB],
                        start=(ec == 0), stop=(ec == EC - 1),
                    )
            dst = {"q": qTs, "k": kTs, "v": vt_sb}[nm]
            if on_act:
                nc.scalar.activation(dst[:, p, c0:c0 + TCH], pp[:],
                                     AF.Identity, bias=b_sb[nm][:, p:p + 1])
            else:
                nc.vector.tensor_scalar_add(dst[:, p, c0:c0 + TCH], pp[:],
                                            b_sb[nm][:, p:p + 1])

        def vtrans2(p, st):
            """transpose vt pair-dims x s-tiles st, st+1 into vnat."""
            for s in (st, st + 1):
                ptp = psum.tile([128, 128], BF16, tag="sc", name="sc")
                nc.tensor.transpose(ptp[:], vt_sb[:, p, s * 128:(s + 1) * 128],
                                    ident[:])
                nc.vector.tensor_copy(vnat[:, p, s, 0, 0:64], ptp[:, 0:64])
                nc.vector.tensor_copy(vnat[:, p, s, 1, 0:64], ptp[:, 64:128])

        def do_item(it):
            if it[0] == "proj":
                _, nm, p, c0, on_act = it
                proj_chunk(nm, p, c0, on_act)
            elif it[0] == "vt2":
                _, p, st = it
                vtrans2(p, st)
            else:
                _, nm, c0 = it
                load_x(nm, c0)

        def attention_block(p, th, weave):
            t0 = th * TCH
            pouts = [psum.tile([65, TCH], F32, tag="acc", name="acc")
                     for _ in range(2)]
            for sp in range(ST // 2):
                for it in (weave[sp] if sp < len(weave) else []):
                    do_item(it)
                pt2, bt2 = [], []
                for h in range(2):
                    pt2.append(ptpool.tile([128, 2 * TCH], BF16, tag="pt",
                                           name="pt"))
                    bt = biasp.tile([128, 2 * TCH], BF16, tag="bias", name="bias")
                    for j in range(2):
                        nc.sync.dma_start(
                            bt[:, j * TCH:(j + 1) * TCH],
                            bias_d[2 * p + h, 2 * sp + j, :, th, :])
                    bt2.append(bt)
                for j in range(2):
                    st = 2 * sp + j
                    psc = [psum.tile([128, TCH], F32, tag="sc", name="sc")
                           for _ in range(2)]
                    for n0 in range(0, TCH, NB):
                        for h in range(2):
                            r0 = 64 * h
                            nc.tensor.matmul(
                                psc[h][:, n0:n0 + NB],
                                kTs[r0:r0 + 64, p, st * 128:(st + 1) * 128],
                                qTs[r0:r0 + 64, p, t0 + n0:t0 + n0 + NB],
                                start=True, stop=True,
                            )
                    for h in range(2):
                        nc.scalar.activation(
                            pt2[h][:, j * TCH:(j + 1) * TCH], psc[h][:], AF.Exp)
                for h in range(2):
                    nc.vector.tensor_mul(pt2[h][:], pt2[h][:], bt2[h][:])
                for h in range(2):
                    for j in range(2):
                        st = 2 * sp + j
                        for n0 in range(0, TCH, NB):
                            nc.tensor.matmul(
                                pouts[h][:, n0:n0 + NB],
                                vnat[:, p, st, h, :],
                                pt2[h][:, j * TCH + n0:j * TCH + n0 + NB],
                                start=(st == 0), stop=(st == ST - 1),
                            )
            # normalization: r = 1/den via fast NR reciprocal; outn = pout * r
            # (den copied via ACT: partition-shifted PSUM reads and PSUM
            #  sources for the custom DVE op are broken on HW)
            for h in range(2):
                den = rdenp.tile([1, TCH], F32, tag="den", name="den")
                nc.scalar.copy(den[:], pouts[h][64:65, :])
                rden = rdenp.tile([1, TCH], F32, tag="rden", name="rden")
                nc.vector.reciprocal_approx_fast(rden[:], den[:])
                rb = normp.tile([64, TCH], F32, tag="rb", name="rb")
                nc.gpsimd.partition_broadcast(rb[:], rden[:])
                if h == 0:
                    nc.vector.tensor_mul(
                        outn[0:64, p, t0:t0 + TCH], pouts[h][0:64, :], rb[:])
                else:
                    po_s = normp.tile([64, TCH], F32, tag="po", name="po")
                    nc.vector.tensor_copy(po_s[:], pouts[h][0:64, :])
                    nc.vector.tensor_mul(
                        outn[64:128, p, t0:t0 + TCH], po_s[:], rb[:])

        def outproj_block(th):
            t0 = th * TCH
            for tt in range(TCH // 128):
                r0 = t0 + tt * 128
                py = psum.tile([128, E], F32, tag="acc", name="acc")
                for n0 in range(0, E, NB):
                    for p in range(NPAIR):
                        nc.tensor.matmul(
                            py[:, n0:n0 + NB],
                            outn[:, p, r0:r0 + 128],
                            wo_s[:, p, n0:n0 + NB],
                            start=(p == 0), stop=(p == NPAIR - 1),
                        )
                ys = ysp.tile([128, E], BF16, tag="ys", name="ys")
                nc.vector.tensor_copy(ys[:], py[:])
                nc.sync.dma_start(y_d[r0:r0 + 128, :], ys[:])

        # ---------------- head: minimal pair-0 prep ----------------
        load_x("q", 0)
        load_x("k", 0)
        load_x("v", 0)
        proj_chunk("q", 0, 0, on_act=True)
        proj_chunk("k", 0, 0, on_act=True)
        proj_chunk("v", 0, 0, on_act=True)
        nc.vector.memset(vnat[:, :, :, :, 64:65], 1.0)
        for st in (0, 2, 4, 6):
            vtrans2(0, st)

        # ---------------- woven attention schedule ----------------
        w00 = [
            [("load", "k", TCH)],
            [("load", "v", TCH), ("proj", "k", 0, TCH, False)],
            [("proj", "v", 0, TCH, False)],
            [("vt2", 0, 8)],
            [("vt2", 0, 10), ("load", "q", TCH)],
            [("vt2", 0, 12)],
            [("vt2", 0, 14)],
            [("proj", "q", 0, TCH, False)],
        ]
        w01 = [
            [("proj", "q", 1, 0, False)],
            [("proj", "q", 1, TCH, False)],
            [("proj", "k", 1, 0, False)],
            [("proj", "k", 1, TCH, False)],
            [("proj", "v", 1, 0, False)],
            [("proj", "v", 1, TCH, False)],
            [("vt2", 1, 0), ("vt2", 1, 2)],
            [("vt2", 1, 4), ("vt2", 1, 6)],
        ]
        w10 = [
            [("vt2", 1, 8)],
            [("vt2", 1, 10)],
            [("vt2", 1, 12)],
            [("vt2", 1, 14)],
        ]
        attention_block(0, 0, w00)
        attention_block(0, 1, w01)
        attention_block(1, 0, w10)
        outproj_block(0)
        attention_block(1, 1, [])
        outproj_block(1)

    nc.compile()
    _MODULES[key] = nc
    return nc


def make_in_maps(query, key, value, spatial_bias, directional_bias,
                 key_padding_mask, attn_mask, Wq, bq, Wk, bk, Wv, bv, Wo, bo):
    scale = D ** -0.5
    qT = [np.ascontiguousarray(query[b].T, dtype=NPBF16) for b in range(B)]
    kT = [np.ascontiguousarray(key[b].T, dtype=NPBF16) for b in range(B)]
    vT = [np.ascontiguousarray(value[b].T, dtype=NPBF16) for b in range(B)]
    pad_any = bool(np.any(key_padding_mask))
    in_maps = []
    for c in range(NCORES):
        b = c // 4
        h0 = (c % 4) * HPC
        sl = slice(h0 * D, (h0 + HPC) * D)
        bias = spatial_bias[b, h0:h0 + HPC].astype(np.float32) \
            + directional_bias[b, h0:h0 + HPC]
        bias += attn_mask[None]
        if pad_any:
            bias = np.where(key_padding_mask[b, None, None, :], -1e30, bias)
        np.exp(bias, out=bias)  # kernel applies bias multiplicatively
        # [h, T, S] -> [h, S, T] -> chunk to [h, st, 128, th, TCH]
        biasT = np.ascontiguousarray(bias.transpose(0, 2, 1), dtype=NPBF16)
        biasT = np.ascontiguousarray(
            biasT.reshape(HPC, ST, 128, NTH, TCH))
        in_maps.append({
            "qT": qT[b], "kT": kT[b], "vT": vT[b], "biasT": biasT,
            "wqT": np.ascontiguousarray((Wq[sl, :].T * scale), dtype=NPBF16),
            "wkT": np.ascontiguousarray(Wk[sl, :].T, dtype=NPBF16),
            "wvT": np.ascontiguousarray(Wv[sl, :].T, dtype=NPBF16),
            "woT": np.ascontiguousarray(Wo[:, sl].T, dtype=NPBF16),
            "bq": (bq[sl] * scale).reshape(NPAIR, 128).T.astype(np.float32).copy(),
            "bk": bk[sl].reshape(NPAIR, 128).T.astype(np.float32).copy(),
            "bv": bv[sl].reshape(NPAIR, 128).T.astype(np.float32).copy(),
        })
    return in_maps


def _install_ntff_shim():
    """bass_utils' trace path imports antenv.axon_hooks, which this image
    lacks; synthesize it around trn_boot's ctypes NTFF hook."""
    import sys
    import types
    if "antenv.axon_hooks" in sys.modules:
        return
    try:
        import antenv
        from trn_agent_boot.trn_boot import _ntff_profile_via_ctypes
        hook = _ntff_profile_via_ctypes("/opt/axon/libaxon_pjrt.so")
        mod = types.ModuleType("antenv.axon_hooks")
        mod._hook = hook
        mod.get_axon_ntff_profile_hook = lambda: mod._hook
        mod.set_axon_ntff_profile_hook = lambda h: setattr(mod, "_hook", h)
        sys.modules["antenv.axon_hooks"] = mod
        antenv.axon_hooks = mod
    except Exception as exc:  # pragma: no cover
        print("ntff shim unavailable:", exc)


def kernel(**inputs):
    global LAST_RUN
    if os.environ.get("BASS_TRACE"):
        _install_ntff_shim()
    nc = build_module()
    in_maps = make_in_maps(**inputs)
    res = run_bass_kernel_spmd(
        nc, in_maps, core_ids=list(range(NCORES)),
        trace=bool(os.environ.get("BASS_TRACE")),
    )
    LAST_RUN = res
    bo = inputs["bo"]
    y = np.zeros((B, T, E), dtype=np.float64)
    for c in range(NCORES):
        y[c // 4] += res.results[c]["ypart"].astype(np.float64)
    y += bo
    return y.astype(np.float32)


# revision 11
# speedup vs baseline: 1.4578x; 1.2126x over previous
"""Graphormer multi-head attention on 8 TRN2 NeuronCores.

Sharding (2D, data + head parallel): core c -> batch c//4, head-quad c%4
(4 heads per core as 2 pairs).  Per-core DMA: q/k/v only for its batch
(12.6 MB), bias slice 33.5 MB bf16, bf16 partial output 4.2 MB.

 - QKV projections column-parallel per pair (128 of 1024 output dims each).
 - Attention in transposed layout: scoresT = K@Q^T with S on partitions and
   T free.  The two heads of a pair use PE row-tiling (K=64 stationaries at
   partitions 0-63 / 64-127 -> tile_position (0,0)/(64,0)) so their scores
   matmuls can overlap in the PE array.
 - Softmax denominator from a ones column appended to the PV stationary
   (row 64 of the PV accumulator); 1/den via the custom-DVE
   reciprocal_approx_fast, broadcast across partitions on idle GPSIMD.
 - bias (spatial+directional+attn_mask, exp'd and bf16 on the host) applied
   multiplicatively on DVE at 2x rate over [128, 2048] tiles.
 - Out-projection column-parallel over this core's 256 dims; bf16 partials
   summed on the host (the all-reduce) together with bo.

Emission is software-pipelined: a minimal head (first 1024-column chunk of
the q/k/v projections + first half of the V transposes for pair 0), then
the attention st-loops with the remaining projection/transpose work woven
one-or-two items per st-pair, so the ACT engine (the exp wall, ~16.8M
elements/core ~= 142us) starts early and never starves.
"""

import os
from contextlib import ExitStack

import ml_dtypes
import numpy as np

import concourse.bass as bass
import concourse.tile as tile
from concourse import bacc
from concourse import mybir
from concourse.bass_utils import run_bass_kernel_spmd
from concourse.masks import make_identity

B, T, S, E, H, D = 2, 2048, 2048, 1024, 16, 64
NCORES = 8
HPC = 4                    # heads per core
NPAIR = 2                  # head pairs per core
PSL = HPC * D              # per-core projection slice = 256
EC = E // 128              # contraction chunks = 8
ST = S // 128              # s tiles = 16
TCH = 1024                 # t block
NTH = T // TCH             # 2
NB = 512                   # fp32 psum bank free size
BF16 = mybir.dt.bfloat16
F32 = mybir.dt.float32
NPBF16 = ml_dtypes.bfloat16
AF = mybir.ActivationFunctionType

_MODULES = {}
LAST_RUN = None


def build_module():
    key = "main"
    if key in _MODULES:
        return _MODULES[key]

    nc = bacc.Bacc("TRN2", target_bir_lowering=False, debug=False)

    qT_d = nc.dram_tensor("qT", [E, T], BF16, kind="ExternalInput")
    kT_d = nc.dram_tensor("kT", [E, S], BF16, kind="ExternalInput")
    vT_d = nc.dram_tensor("vT", [E, S], BF16, kind="ExternalInput")
    # host layout: [head, st, 128, th, TCH] (exp'd bias, transposed (s,t))
    bias_d = nc.dram_tensor("biasT", [HPC, ST, 128, NTH, TCH], BF16,
                            kind="ExternalInput")
    wq_d = nc.dram_tensor("wqT", [E, PSL], BF16, kind="ExternalInput")
    wk_d = nc.dram_tensor("wkT", [E, PSL], BF16, kind="ExternalInput")
    wv_d = nc.dram_tensor("wvT", [E, PSL], BF16, kind="ExternalInput")
    wo_d = nc.dram_tensor("woT", [PSL, E], BF16, kind="ExternalInput")
    bq_d = nc.dram_tensor("bq", [128, NPAIR], F32, kind="ExternalInput")
    bk_d = nc.dram_tensor("bk", [128, NPAIR], F32, kind="ExternalInput")
    bv_d = nc.dram_tensor("bv", [128, NPAIR], F32, kind="ExternalInput")
    y_d = nc.dram_tensor("ypart", [T, E], BF16, kind="ExternalOutput")

    with tile.TileContext(nc) as tc, ExitStack() as ctx:
        consts = ctx.enter_context(tc.tile_pool(name="consts", bufs=1))
        xpool = ctx.enter_context(tc.tile_pool(name="xstage", bufs=1))
        persist = ctx.enter_context(tc.tile_pool(name="persist", bufs=1))
        biasp = ctx.enter_context(tc.tile_pool(name="biasp", bufs=3))
        ptpool = ctx.enter_context(tc.tile_pool(name="ptpool", bufs=4))
        normp = ctx.enter_context(tc.tile_pool(name="normp", bufs=1))
        rdenp = ctx.enter_context(tc.tile_pool(name="rdenp", bufs=1))
        ysp = ctx.enter_context(tc.tile_pool(name="ysp", bufs=2))
        # psum: scores/proj chunks [128,512] f32 = 1 bank x4; acc = 2 banks x2
        psumS = ctx.enter_context(tc.tile_pool(name="psumS", bufs=4, space="PSUM"))
        psumA = ctx.enter_context(tc.tile_pool(name="psumA", bufs=2, space="PSUM"))

        ident = consts.tile([128, 128], BF16, tag="ident", name="ident")
        make_identity(nc, ident[:])
        w_sb = {}
        for nm, wd in (("q", wq_d), ("k", wk_d), ("v", wv_d)):
            w_s = consts.tile([128, EC, PSL], BF16, tag=f"w{nm}", name=f"w{nm}")
            for ec in range(EC):
                nc.sync.dma_start(w_s[:, ec, :], wd[ec * 128:(ec + 1) * 128, :])
            w_sb[nm] = w_s
        wo_s = consts.tile([128, NPAIR, E], BF16, tag="wo", name="wo")
        for p in range(NPAIR):
            nc.sync.dma_start(wo_s[:, p, :], wo_d[p * 128:(p + 1) * 128, :])
        b_sb = {}
        for nm, bd in (("q", bq_d), ("k", bk_d), ("v", bv_d)):
            b_s = consts.tile([128, NPAIR], F32, tag=f"b{nm}", name=f"b{nm}")
            nc.sync.dma_start(b_s[:], bd[:])
            b_sb[nm] = b_s

        x_sb, x_d = {}, {"q": qT_d, "k": kT_d, "v": vT_d}
        for nm, L in (("q", T), ("k", S), ("v", S)):
            x_sb[nm] = xpool.tile([128, EC, L], BF16, tag=f"x{nm}", name=f"x{nm}")

        qTs = persist.tile([128, NPAIR, T], BF16, tag="qTs", name="qTs")
        kTs = persist.tile([128, NPAIR, S], BF16, tag="kTs", name="kTs")
        # [s-part, pair, st, head-in-pair, d | ones col]
        vnat = persist.tile([128, NPAIR, ST, 2, 65], BF16, tag="vnat", name="vnat")
        outn = persist.tile([128, NPAIR, T], BF16, tag="outn", name="outn")
        vt_sb = persist.tile([128, NPAIR, S], BF16, tag="vt", name="vt")

        def load_x(nm, c0):
            for ec in range(EC):
                nc.sync.dma_start(x_sb[nm][:, ec, c0:c0 + TCH],
                                  x_d[nm][ec * 128:(ec + 1) * 128, c0:c0 + TCH])

        def proj_chunk(nm, p, c0, on_act):
            """project x[nm] cols [c0, c0+1024) for pair p."""
            dst = {"q": qTs, "k": kTs, "v": vt_sb}[nm]
            for n0 in range(0, TCH, NB):
                pp = psumS.tile([128, NB], F32, tag="sc", name="sc")
                for ec in range(EC):
                    nc.tensor.matmul(
                        pp[:],
                        w_sb[nm][:, ec, p * 128:(p + 1) * 128],
                        x_sb[nm][:, ec, c0 + n0:c0 + n0 + NB],
                        start=(ec == 0), stop=(ec == EC - 1),
                    )
                if on_act:
                    nc.scalar.activation(dst[:, p, c0 + n0:c0 + n0 + NB], pp[:],
                                         AF.Identity, bias=b_sb[nm][:, p:p + 1])
                else:
                    nc.vector.tensor_scalar_add(
                        dst[:, p, c0 + n0:c0 + n0 + NB], pp[:],
                        b_sb[nm][:, p:p + 1])

        def vtrans2(p, st):
            """transpose vt pair-dims x s-tiles st, st+1 into vnat."""
            for s in (st, st + 1):
                ptp = psumS.tile([128, 128], BF16, tag="sc", name="sc")
                nc.tensor.transpose(ptp[:], vt_sb[:, p, s * 128:(s + 1) * 128],
                                    ident[:])
                nc.vector.tensor_copy(vnat[:, p, s, 0, 0:64], ptp[:, 0:64])
                nc.vector.tensor_copy(vnat[:, p, s, 1, 0:64], ptp[:, 64:128])

        def do_item(it):
            if it[0] == "proj":
                _, nm, p, c0, on_act = it
                proj_chunk(nm, p, c0, on_act)
            elif it[0] == "vt2":
                _, p, st = it
                vtrans2(p, st)
            else:
                _, nm, c0 = it
                load_x(nm, c0)

        def attention_block(p, th, weave):
            t0 = th * TCH
            pouts = [psumA.tile([65, TCH], F32, tag="acc", name="acc")
                     for _ in range(2)]

            def emit_pv(pt2_, sp_):
                for h in range(2):
                    for j in range(2):
                        st = 2 * sp_ + j
                        for n0 in range(0, TCH, NB):
                            nc.tensor.matmul(
                                pouts[h][:, n0:n0 + NB],
                                vnat[:, p, st, h, :],
                                pt2_[h][:, j * TCH + n0:j * TCH + n0 + NB],
                                start=(st == 0), stop=(st == ST - 1),
                            )

            pending = None
            for sp in range(ST // 2):
                for it in (weave[sp] if sp < len(weave) else []):
                    do_item(it)
                pt2, bt2 = [], []
                for h in range(2):
                    pt2.append(ptpool.tile([128, 2 * TCH], BF16, tag="pt",
                                           name="pt"))
                    bt = biasp.tile([128, 2 * TCH], BF16, tag="bias", name="bias")
                    for j in range(2):
                        nc.sync.dma_start(
                            bt[:, j * TCH:(j + 1) * TCH],
                            bias_d[2 * p + h, 2 * sp + j, :, th, :])
                    bt2.append(bt)
                for h in range(2):
                    r0 = 64 * h
                    for j in range(2):
                        st = 2 * sp + j
                        for n0 in range(0, TCH, NB):
                            psc = psumS.tile([128, NB], F32, tag="sc", name="sc")
                            nc.tensor.matmul(
                                psc[:],
                                kTs[r0:r0 + 64, p, st * 128:(st + 1) * 128],
                                qTs[r0:r0 + 64, p, t0 + n0:t0 + n0 + NB],
                                start=True, stop=True,
                            )
                            nc.scalar.activation(
                                pt2[h][:, j * TCH + n0:j * TCH + n0 + NB],
                                psc[:], AF.Exp)
                # previous st-pair's PV goes here: its bias-mult overlaps
                # this sp's scores, so the PE never waits on DVE
                if pending is not None:
                    emit_pv(*pending)
                for h in range(2):
                    nc.vector.tensor_mul(pt2[h][:], pt2[h][:], bt2[h][:])
                pending = (pt2, sp)
            emit_pv(*pending)
            # normalization: r = 1/den via fast NR reciprocal; outn = pout * r
            # (den copied via ACT: partition-shifted PSUM reads and PSUM
            #  sources for the custom DVE op are broken on HW)
            for h in range(2):
                den = rdenp.tile([1, TCH], F32, tag="den", name="den")
                nc.scalar.copy(den[:], pouts[h][64:65, :])
                # (pouts from psumA)
                rden = rdenp.tile([1, TCH], F32, tag="rden", name="rden")
                nc.vector.reciprocal_approx_fast(rden[:], den[:])
                rb = normp.tile([64, TCH], F32, tag="rb", name="rb")
                nc.gpsimd.partition_broadcast(rb[:], rden[:])
                if h == 0:
                    nc.vector.tensor_mul(
                        outn[0:64, p, t0:t0 + TCH], pouts[h][0:64, :], rb[:])
                else:
                    po_s = normp.tile([64, TCH], F32, tag="po", name="po")
                    nc.vector.tensor_copy(po_s[:], pouts[h][0:64, :])
                    nc.vector.tensor_mul(
                        outn[64:128, p, t0:t0 + TCH], po_s[:], rb[:])

        def outproj_block(th):
            t0 = th * TCH
            for tt in range(TCH // 128):
                r0 = t0 + tt * 128
                py = psumA.tile([128, E], F32, tag="acc", name="acc")
                for n0 in range(0, E, NB):
                    for p in range(NPAIR):
                        nc.tensor.matmul(
                            py[:, n0:n0 + NB],
                            outn[:, p, r0:r0 + 128],
                            wo_s[:, p, n0:n0 + NB],
                            start=(p == 0), stop=(p == NPAIR - 1),
                        )
                ys = ysp.tile([128, E], BF16, tag="ys", name="ys")
                nc.vector.tensor_copy(ys[:], py[:])
                nc.sync.dma_start(y_d[r0:r0 + 128, :], ys[:])

        # ---------------- head: minimal pair-0 prep ----------------
        load_x("q", 0)
        load_x("k", 0)
        load_x("v", 0)
        proj_chunk("q", 0, 0, on_act=True)
        proj_chunk("k", 0, 0, on_act=True)
        proj_chunk("v", 0, 0, on_act=True)
        nc.vector.memset(vnat[:, :, :, :, 64:65], 1.0)
        for st in (0, 2, 4, 6):
            vtrans2(0, st)

        # ---------------- woven attention schedule ----------------
        w00 = [
            [("load", "k", TCH)],
            [("load", "v", TCH), ("proj", "k", 0, TCH, False)],
            [("proj", "v", 0, TCH, False)],
            [("vt2", 0, 8)],
            [("vt2", 0, 10), ("load", "q", TCH)],
            [("vt2", 0, 12)],
            [("vt2", 0, 14)],
            [("proj", "q", 0, TCH, False)],
        ]
        w01 = [
            [("proj", "q", 1, 0, False)],
            [("proj", "q", 1, TCH, False)],
            [("proj", "k", 1, 0, False)],
            [("proj", "k", 1, TCH, False)],
            [("proj", "v", 1, 0, False)],
            [("proj", "v", 1, TCH, False)],
            [("vt2", 1, 0), ("vt2", 1, 2)],
            [("vt2", 1, 4), ("vt2", 1, 6)],
        ]
        w10 = [
            [("vt2", 1, 8)],
            [("vt2", 1, 10)],
            [("vt2", 1, 12)],
            [("vt2", 1, 14)],
        ]
        attention_block(0, 0, w00)
        attention_block(0, 1, w01)
        attention_block(1, 0, w10)
        outproj_block(0)
        attention_block(1, 1, [])
        outproj_block(1)

    nc.compile()
    _MODULES[key] = nc
    return nc


def make_in_maps(query, key, value, spatial_bias, directional_bias,
                 key_padding_mask, attn_mask, Wq, bq, Wk, bk, Wv, bv, Wo, bo):
    scale = D ** -0.5
    qT = [np.ascontiguousarray(query[b].T, dtype=NPBF16) for b in range(B)]
    kT = [np.ascontiguousarray(key[b].T, dtype=NPBF16) for b in range(B)]
    vT = [np.ascontiguousarray(value[b].T, dtype=NPBF16) for b in range(B)]
    pad_any = bool(np.any(key_padding_mask))
    in_maps = []
    for c in range(NCORES):
        b = c // 4
        h0 = (c % 4) * HPC
        sl = slice(h0 * D, (h0 + HPC) * D)
        bias = spatial_bias[b, h0:h0 + HPC].astype(np.float32) \
            + directional_bias[b, h0:h0 + HPC]
        bias += attn_mask[None]
        if pad_any:
            bias = np.where(key_padding_mask[b, None, None, :], -1e30, bias)
        np.exp(bias, out=bias)  # kernel applies bias multiplicatively
        # [h, T, S] -> [h, S, T] -> chunk to [h, st, 128, th, TCH]
        biasT = np.ascontiguousarray(bias.transpose(0, 2, 1), dtype=NPBF16)
        biasT = np.ascontiguousarray(
            biasT.reshape(HPC, ST, 128, NTH, TCH))
        in_maps.append({
            "qT": qT[b], "kT": kT[b], "vT": vT[b], "biasT": biasT,
            "wqT": np.ascontiguousarray((Wq[sl, :].T * scale), dtype=NPBF16),
            "wkT": np.ascontiguousarray(Wk[sl, :].T, dtype=NPBF16),
            "wvT": np.ascontiguousarray(Wv[sl, :].T, dtype=NPBF16),
            "woT": np.ascontiguousarray(Wo[:, sl].T, dtype=NPBF16),
            "bq": (bq[sl] * scale).reshape(NPAIR, 128).T.astype(np.float32).copy(),
            "bk": bk[sl].reshape(NPAIR, 128).T.astype(np.float32).copy(),
            "bv": bv[sl].reshape(NPAIR, 128).T.astype(np.float32).copy(),
        })
    return in_maps


def _install_ntff_shim():
    """bass_utils' trace path imports antenv.axon_hooks, which this image
    lacks; synthesize it around trn_boot's ctypes NTFF hook."""
    import sys
    import types
    if "antenv.axon_hooks" in sys.modules:
        return
    try:
        import antenv
        from trn_agent_boot.trn_boot import _ntff_profile_via_ctypes
        hook = _ntff_profile_via_ctypes("/opt/axon/libaxon_pjrt.so")
        mod = types.ModuleType("antenv.axon_hooks")
        mod._hook = hook
        mod.get_axon_ntff_profile_hook = lambda: mod._hook
        mod.set_axon_ntff_profile_hook = lambda h: setattr(mod, "_hook", h)
        sys.modules["antenv.axon_hooks"] = mod
        antenv.axon_hooks = mod
    except Exception as exc:  # pragma: no cover
        print("ntff shim unavailable:", exc)


def kernel(**inputs):
    global LAST_RUN
    if os.environ.get("BASS_TRACE"):
        _install_ntff_shim()
    nc = build_module()
    in_maps = make_in_maps(**inputs)
    res = run_bass_kernel_spmd(
        nc, in_maps, core_ids=list(range(NCORES)),
        trace=bool(os.environ.get("BASS_TRACE")),
    )
    LAST_RUN = res
    bo = inputs["bo"]
    y = np.zeros((B, T, E), dtype=np.float64)
    for c in range(NCORES):
        y[c // 4] += res.results[c]["ypart"].astype(np.float64)
    y += bo
    return y.astype(np.float32)


# revision 12
# speedup vs baseline: 1.5033x; 1.0312x over previous
"""Graphormer multi-head attention on 8 TRN2 NeuronCores.

Sharding (2D, data + head parallel): core c -> batch c//4, head-quad c%4
(4 heads per core as 2 pairs).  Per-core DMA: q/k/v only for its batch
(12.6 MB), bias slice 33.5 MB bf16, bf16 partial output 4.2 MB.

 - QKV projections column-parallel per pair (128 of 1024 output dims each).
 - Attention in transposed layout: scoresT = K@Q^T with S on partitions and
   T free.  The two heads of a pair use PE row-tiling (K=64 stationaries at
   partitions 0-63 / 64-127 -> tile_position (0,0)/(64,0)) so their scores
   matmuls can overlap in the PE array.
 - Softmax denominator from a ones column appended to the PV stationary
   (row 64 of the PV accumulator); 1/den via the custom-DVE
   reciprocal_approx_fast, broadcast across partitions on idle GPSIMD.
 - bias (spatial+directional+attn_mask, exp'd and bf16 on the host) applied
   multiplicatively on DVE at 2x rate over [128, 2048] tiles.
 - Out-projection column-parallel over this core's 256 dims; bf16 partials
   summed on the host (the all-reduce) together with bo.

Emission is software-pipelined: a minimal head (first 1024-column chunk of
the q/k/v projections + first half of the V transposes for pair 0), then
the attention st-loops with the remaining projection/transpose work woven
one-or-two items per st-pair, so the ACT engine (the exp wall, ~16.8M
elements/core ~= 142us) starts early and never starves.
"""

import os
from contextlib import ExitStack

import ml_dtypes
import numpy as np

import concourse.bass as bass
import concourse.tile as tile
from concourse import bacc
from concourse import mybir
from concourse.bass_utils import run_bass_kernel_spmd
from concourse.masks import make_identity

B, T, S, E, H, D = 2, 2048, 2048, 1024, 16, 64
NCORES = 8
HPC = 4                    # heads per core
NPAIR = 2                  # head pairs per core
PSL = HPC * D              # per-core projection slice = 256
EC = E // 128              # contraction chunks = 8
ST = S // 128              # s tiles = 16
TCH = 1024                 # t block
NTH = T // TCH             # 2
NB = 512                   # fp32 psum bank free size
BF16 = mybir.dt.bfloat16
F32 = mybir.dt.float32
NPBF16 = ml_dtypes.bfloat16
AF = mybir.ActivationFunctionType

_MODULES = {}
LAST_RUN = None


def build_module():
    key = "main"
    if key in _MODULES:
        return _MODULES[key]

    nc = bacc.Bacc("TRN2", target_bir_lowering=False, debug=False)

    qT_d = nc.dram_tensor("qT", [E, T], BF16, kind="ExternalInput")
    kT_d = nc.dram_tensor("kT", [E, S], BF16, kind="ExternalInput")
    vT_d = nc.dram_tensor("vT", [E, S], BF16, kind="ExternalInput")
    # host layout: [head, st, 128, th, TCH] (exp'd bias, transposed (s,t))
    bias_d = nc.dram_tensor("biasT", [HPC, ST, 128, NTH, TCH], BF16,
                            kind="ExternalInput")
    wq_d = nc.dram_tensor("wqT", [E, PSL], BF16, kind="ExternalInput")
    wk_d = nc.dram_tensor("wkT", [E, PSL], BF16, kind="ExternalInput")
    wv_d = nc.dram_tensor("wvT", [E, PSL], BF16, kind="ExternalInput")
    wo_d = nc.dram_tensor("woT", [PSL, E], BF16, kind="ExternalInput")
    bq_d = nc.dram_tensor("bq", [128, NPAIR], F32, kind="ExternalInput")
    bk_d = nc.dram_tensor("bk", [128, NPAIR], F32, kind="ExternalInput")
    bv_d = nc.dram_tensor("bv", [128, NPAIR], F32, kind="ExternalInput")
    y_d = nc.dram_tensor("ypart", [T, E], BF16, kind="ExternalOutput")

    with tile.TileContext(nc) as tc, ExitStack() as ctx:
        consts = ctx.enter_context(tc.tile_pool(name="consts", bufs=1))
        xpool = ctx.enter_context(tc.tile_pool(name="xstage", bufs=1))
        persist = ctx.enter_context(tc.tile_pool(name="persist", bufs=1))
        biasp = ctx.enter_context(tc.tile_pool(name="biasp", bufs=3))
        ptpool = ctx.enter_context(tc.tile_pool(name="ptpool", bufs=4))
        normp = ctx.enter_context(tc.tile_pool(name="normp", bufs=1))
        rdenp = ctx.enter_context(tc.tile_pool(name="rdenp", bufs=1))
        ysp = ctx.enter_context(tc.tile_pool(name="ysp", bufs=2))
        # psum: scores/proj chunks [128,512] f32 = 1 bank x4; acc = 2 banks x2
        psumS = ctx.enter_context(tc.tile_pool(name="psumS", bufs=4, space="PSUM"))
        psumA = ctx.enter_context(tc.tile_pool(name="psumA", bufs=2, space="PSUM"))

        ident = consts.tile([128, 128], BF16, tag="ident", name="ident")
        make_identity(nc, ident[:])
        w_sb = {}
        for nm, wd in (("q", wq_d), ("k", wk_d), ("v", wv_d)):
            w_s = consts.tile([128, EC, PSL], BF16, tag=f"w{nm}", name=f"w{nm}")
            for ec in range(EC):
                nc.sync.dma_start(w_s[:, ec, :], wd[ec * 128:(ec + 1) * 128, :])
            w_sb[nm] = w_s
        wo_s = consts.tile([128, NPAIR, E], BF16, tag="wo", name="wo")
        for p in range(NPAIR):
            nc.sync.dma_start(wo_s[:, p, :], wo_d[p * 128:(p + 1) * 128, :])
        b_sb = {}
        for nm, bd in (("q", bq_d), ("k", bk_d), ("v", bv_d)):
            b_s = consts.tile([128, NPAIR], F32, tag=f"b{nm}", name=f"b{nm}")
            nc.sync.dma_start(b_s[:], bd[:])
            b_sb[nm] = b_s

        x_sb, x_d = {}, {"q": qT_d, "k": kT_d, "v": vT_d}
        for nm, L in (("q", T), ("k", S), ("v", S)):
            x_sb[nm] = xpool.tile([128, EC, L], BF16, tag=f"x{nm}", name=f"x{nm}")

        qTs = persist.tile([128, NPAIR, T], BF16, tag="qTs", name="qTs")
        kTs = persist.tile([128, NPAIR, S], BF16, tag="kTs", name="kTs")
        # [s-part, pair, st, head-in-pair, d | ones col]
        vnat = persist.tile([128, NPAIR, ST, 2, 65], BF16, tag="vnat", name="vnat")
        outn = persist.tile([128, NPAIR, T], BF16, tag="outn", name="outn")
        vt_sb = persist.tile([128, NPAIR, S], BF16, tag="vt", name="vt")

        def load_x(nm, c0):
            for ec in range(EC):
                nc.sync.dma_start(x_sb[nm][:, ec, c0:c0 + TCH],
                                  x_d[nm][ec * 128:(ec + 1) * 128, c0:c0 + TCH])

        def proj_chunk(nm, p, c0, on_act):
            """project x[nm] cols [c0, c0+1024) for pair p."""
            dst = {"q": qTs, "k": kTs, "v": vt_sb}[nm]
            for n0 in range(0, TCH, NB):
                pp = psumS.tile([128, NB], F32, tag="sc", name="sc")
                for ec in range(EC):
                    nc.tensor.matmul(
                        pp[:],
                        w_sb[nm][:, ec, p * 128:(p + 1) * 128],
                        x_sb[nm][:, ec, c0 + n0:c0 + n0 + NB],
                        start=(ec == 0), stop=(ec == EC - 1),
                    )
                if on_act:
                    nc.scalar.activation(dst[:, p, c0 + n0:c0 + n0 + NB], pp[:],
                                         AF.Identity, bias=b_sb[nm][:, p:p + 1])
                else:
                    nc.vector.tensor_scalar_add(
                        dst[:, p, c0 + n0:c0 + n0 + NB], pp[:],
                        b_sb[nm][:, p:p + 1])

        def vtrans2(p, st):
            """transpose vt pair-dims x s-tiles st, st+1 into vnat."""
            for s in (st, st + 1):
                ptp = psumS.tile([128, 128], BF16, tag="sc", name="sc")
                nc.tensor.transpose(ptp[:], vt_sb[:, p, s * 128:(s + 1) * 128],
                                    ident[:])
                nc.vector.tensor_copy(vnat[:, p, s, 0, 0:64], ptp[:, 0:64])
                nc.vector.tensor_copy(vnat[:, p, s, 1, 0:64], ptp[:, 64:128])

        def do_item(it):
            if it[0] == "proj":
                _, nm, p, c0, on_act = it
                proj_chunk(nm, p, c0, on_act)
            elif it[0] == "vt2":
                _, p, st = it
                vtrans2(p, st)
            else:
                _, nm, c0 = it
                load_x(nm, c0)

        def attention_block(p, th, weave):
            t0 = th * TCH
            pouts = [psumA.tile([65, TCH], F32, tag="acc", name="acc")
                     for _ in range(2)]

            def emit_pv(pt2_, sp_):
                for h in range(2):
                    for j in range(2):
                        st = 2 * sp_ + j
                        for n0 in range(0, TCH, NB):
                            nc.tensor.matmul(
                                pouts[h][:, n0:n0 + NB],
                                vnat[:, p, st, h, :],
                                pt2_[h][:, j * TCH + n0:j * TCH + n0 + NB],
                                start=(st == 0), stop=(st == ST - 1),
                            )

            pending = None
            for sp in range(ST // 2):
                for it in (weave[sp] if sp < len(weave) else []):
                    do_item(it)
                pt2, bt2 = [], []
                for h in range(2):
                    pt2.append(ptpool.tile([128, 2 * TCH], BF16, tag="pt",
                                           name="pt"))
                    bt = biasp.tile([128, 2 * TCH], BF16, tag="bias", name="bias")
                    for j in range(2):
                        nc.sync.dma_start(
                            bt[:, j * TCH:(j + 1) * TCH],
                            bias_d[2 * p + h, 2 * sp + j, :, th, :])
                    bt2.append(bt)
                for j in range(2):
                    st = 2 * sp + j
                    for n0 in range(0, TCH, NB):
                        # the two heads' K=64 matmuls sit in different PE
                        # row-groups and are emitted back-to-back: they
                        # dual-stream (measured ~2x on HW)
                        pscs = []
                        for h in range(2):
                            r0 = 64 * h
                            psc = psumS.tile([128, NB], F32, tag="sc", name="sc")
                            nc.tensor.matmul(
                                psc[:],
                                kTs[r0:r0 + 64, p, st * 128:(st + 1) * 128],
                                qTs[r0:r0 + 64, p, t0 + n0:t0 + n0 + NB],
                                start=True, stop=True,
                                tile_position=(r0, 0),
                            )
                            pscs.append(psc)
                        for h in range(2):
                            nc.scalar.activation(
                                pt2[h][:, j * TCH + n0:j * TCH + n0 + NB],
                                pscs[h][:], AF.Exp)
                # previous st-pair's PV goes here: its bias-mult overlaps
                # this sp's scores, so the PE never waits on DVE
                if pending is not None:
                    emit_pv(*pending)
                for h in range(2):
                    nc.vector.tensor_mul(pt2[h][:], pt2[h][:], bt2[h][:])
                pending = (pt2, sp)
            emit_pv(*pending)
            # normalization: r = 1/den via fast NR reciprocal; outn = pout * r
            # (den copied via ACT: partition-shifted PSUM reads and PSUM
            #  sources for the custom DVE op are broken on HW)
            for h in range(2):
                den = rdenp.tile([1, TCH], F32, tag="den", name="den")
                nc.scalar.copy(den[:], pouts[h][64:65, :])
                # (pouts from psumA)
                rden = rdenp.tile([1, TCH], F32, tag="rden", name="rden")
                nc.vector.reciprocal_approx_fast(rden[:], den[:])
                rb = normp.tile([64, TCH], F32, tag="rb", name="rb")
                nc.gpsimd.partition_broadcast(rb[:], rden[:])
                if h == 0:
                    nc.vector.tensor_mul(
                        outn[0:64, p, t0:t0 + TCH], pouts[h][0:64, :], rb[:])
                else:
                    po_s = normp.tile([64, TCH], F32, tag="po", name="po")
                    nc.vector.tensor_copy(po_s[:], pouts[h][0:64, :])
                    nc.vector.tensor_mul(
                        outn[64:128, p, t0:t0 + TCH], po_s[:], rb[:])

        def outproj_block(th):
            t0 = th * TCH
            for tt in range(TCH // 128):
                r0 = t0 + tt * 128
                py = psumA.tile([128, E], F32, tag="acc", name="acc")
                for n0 in range(0, E, NB):
                    for p in range(NPAIR):
                        nc.tensor.matmul(
                            py[:, n0:n0 + NB],
                            outn[:, p, r0:r0 + 128],
                            wo_s[:, p, n0:n0 + NB],
                            start=(p == 0), stop=(p == NPAIR - 1),
                        )
                ys = ysp.tile([128, E], BF16, tag="ys", name="ys")
                nc.vector.tensor_copy(ys[:], py[:])
                nc.sync.dma_start(y_d[r0:r0 + 128, :], ys[:])

        # ---------------- head: minimal pair-0 prep ----------------
        load_x("q", 0)
        load_x("k", 0)
        load_x("v", 0)
        proj_chunk("q", 0, 0, on_act=True)
        proj_chunk("k", 0, 0, on_act=True)
        proj_chunk("v", 0, 0, on_act=True)
        nc.vector.memset(vnat[:, :, :, :, 64:65], 1.0)
        for st in (0, 2, 4, 6):
            vtrans2(0, st)

        # ---------------- woven attention schedule ----------------
        w00 = [
            [("load", "k", TCH)],
            [("load", "v", TCH), ("proj", "k", 0, TCH, False)],
            [("proj", "v", 0, TCH, False)],
            [("vt2", 0, 8)],
            [("vt2", 0, 10), ("load", "q", TCH)],
            [("vt2", 0, 12)],
            [("vt2", 0, 14)],
            [("proj", "q", 0, TCH, False)],
        ]
        w01 = [
            [("proj", "q", 1, 0, False)],
            [("proj", "q", 1, TCH, False)],
            [("proj", "k", 1, 0, False)],
            [("proj", "k", 1, TCH, False)],
            [("proj", "v", 1, 0, False)],
            [("proj", "v", 1, TCH, False)],
            [("vt2", 1, 0), ("vt2", 1, 2)],
            [("vt2", 1, 4), ("vt2", 1, 6)],
        ]
        w10 = [
            [("vt2", 1, 8)],
            [("vt2", 1, 10)],
            [("vt2", 1, 12)],
            [("vt2", 1, 14)],
        ]
        attention_block(0, 0, w00)
        attention_block(0, 1, w01)
        attention_block(1, 0, w10)
        outproj_block(0)
        attention_block(1, 1, [])
        outproj_block(1)

    nc.compile()
    _MODULES[key] = nc
    return nc


def make_in_maps(query, key, value, spatial_bias, directional_bias,
                 key_padding_mask, attn_mask, Wq, bq, Wk, bk, Wv, bv, Wo, bo):
    scale = D ** -0.5
    qT = [np.ascontiguousarray(query[b].T, dtype=NPBF16) for b in range(B)]
    kT = [np.ascontiguousarray(key[b].T, dtype=NPBF16) for b in range(B)]
    vT = [np.ascontiguousarray(value[b].T, dtype=NPBF16) for b in range(B)]
    pad_any = bool(np.any(key_padding_mask))
    in_maps = []
    for c in range(NCORES):
        b = c // 4
        h0 = (c % 4) * HPC
        sl = slice(h0 * D, (h0 + HPC) * D)
        bias = spatial_bias[b, h0:h0 + HPC].astype(np.float32) \
            + directional_bias[b, h0:h0 + HPC]
        bias += attn_mask[None]
        if pad_any:
            bias = np.where(key_padding_mask[b, None, None, :], -1e30, bias)
        np.exp(bias, out=bias)  # kernel applies bias multiplicatively
        # [h, T, S] -> [h, S, T] -> chunk to [h, st, 128, th, TCH]
        biasT = np.ascontiguousarray(bias.transpose(0, 2, 1), dtype=NPBF16)
        biasT = np.ascontiguousarray(
            biasT.reshape(HPC, ST, 128, NTH, TCH))
        in_maps.append({
            "qT": qT[b], "kT": kT[b], "vT": vT[b], "biasT": biasT,
            "wqT": np.ascontiguousarray((Wq[sl, :].T * scale), dtype=NPBF16),
            "wkT": np.ascontiguousarray(Wk[sl, :].T, dtype=NPBF16),
            "wvT": np.ascontiguousarray(Wv[sl, :].T, dtype=NPBF16),
            "woT": np.ascontiguousarray(Wo[:, sl].T, dtype=NPBF16),
            "bq": (bq[sl] * scale).reshape(NPAIR, 128).T.astype(np.float32).copy(),
            "bk": bk[sl].reshape(NPAIR, 128).T.astype(np.float32).copy(),
            "bv": bv[sl].reshape(NPAIR, 128).T.astype(np.float32).copy(),
        })
    return in_maps


def _install_ntff_shim():
    """bass_utils' trace path imports antenv.axon_hooks, which this image
    lacks; synthesize it around trn_boot's ctypes NTFF hook."""
    import sys
    import types
    if "antenv.axon_hooks" in sys.modules:
        return
    try:
        import antenv
        from trn_agent_boot.trn_boot import _ntff_profile_via_ctypes
        hook = _ntff_profile_via_ctypes("/opt/axon/libaxon_pjrt.so")
        mod = types.ModuleType("antenv.axon_hooks")
        mod._hook = hook
        mod.get_axon_ntff_profile_hook = lambda: mod._hook
        mod.set_axon_ntff_profile_hook = lambda h: setattr(mod, "_hook", h)
        sys.modules["antenv.axon_hooks"] = mod
        antenv.axon_hooks = mod
    except Exception as exc:  # pragma: no cover
        print("ntff shim unavailable:", exc)


def kernel(**inputs):
    global LAST_RUN
    if os.environ.get("BASS_TRACE"):
        _install_ntff_shim()
    nc = build_module()
    in_maps = make_in_maps(**inputs)
    res = run_bass_kernel_spmd(
        nc, in_maps, core_ids=list(range(NCORES)),
        trace=bool(os.environ.get("BASS_TRACE")),
    )
    LAST_RUN = res
    bo = inputs["bo"]
    y = np.zeros((B, T, E), dtype=np.float64)
    for c in range(NCORES):
        y[c // 4] += res.results[c]["ypart"].astype(np.float64)
    y += bo
    return y.astype(np.float32)
